# revision 6
# baseline (speedup 1.0000x reference)
"""Trainium2 Bass kernel for nn_DecoderBlock_85761906966851 (v2, pipelined).

The reference decoder block's attention einsum ('bhss,bshd->bshd') takes the
DIAGONAL of the attention matrix, so token i only needs
    diag_prob_i[h] = exp(s_ii) / sum_{j<=i} exp(s_ij)
per head.  The kernel computes causal row-sums of exp(QK^T) (fused
exp+row-accumulate on the scalar engine), diagonal scores via an elementwise
q*k reduction, then a dense per-token tail (Wo projection, LayerNorm, FFN,
LayerNorm).

v2 restructure vs baseline:
  * Software-pipelined: the per-token tail for token-slots 0-1 (Wo, LN1,
    FFN, LN2, store) is interleaved into the exp-bound score phases of
    slots 2-3, so the PE works while ACT grinds exps.
  * bf16 matmul operands everywhere except the residual path (f32r),
    halving DMA bytes and enabling FWL weight loads.
  * Staircase causal mask added in PSUM by an identity matmul on the PE
    (frees the DVE); single exp per (slot, head) spanning all key chunks.
  * LayerNorm rsqrt via exp(-0.5*ln(var+eps)) -- Ln/Exp/Identity live in
    one ACT table set, so zero table reloads even with LNs interleaved
    between score exps.
  * gamma1/beta1/b1 folded into W1/b1 host-side; x+bo residual pre-added
    host-side and DMA'd in row layout (no PE transposes for it).

Sharding: 8 cores = 2 batches x 4 stride offsets; core (b, p) owns tokens
p::4 of batch b (the interleave equalizes causal work).  No collectives;
k is recomputed per core.
"""

import numpy as np
import ml_dtypes

B, S, D, H, FF = 2, 2048, 512, 8, 2048
DK = D // H          # 64
P = 128
NT = 512             # tokens per core
NSLOT = 4
DO = D // P          # 4
KI = D // P          # 4
NFT = FF // P        # 16
EPS = 1e-3
NEG = -1.0e30

bf16 = ml_dtypes.bfloat16

# packed f32 consts: eps | bq(4) | bk(4) | b1(16) | keep(4)
CF_EPS, CF_BQ, CF_BK, CF_B1, CF_KEEP = 0, 1, 5, 9, 25
CF = 29
# broadcast f32 consts [P, 5*D]
BCN = ["bv", "g1", "b1b2", "g2", "be2"]
# bf16 consts: identb(128) | mask(512) | osel(32)
CB_ID, CB_MASK, CB_OSEL = 0, 128, 640
CB = 672

TRACE = False
LAST_EXEC_NS = None
_CACHE = {}


def to_f32r(a):
    """Round fp32 to fp32r (11-bit mantissa, round half up at bit 12)."""
    b = np.ascontiguousarray(a, dtype=np.float32).view(np.uint32)
    r = ((b.astype(np.uint64) + 0x800) & 0xFFFFF000).astype(np.uint32)
    return r.view(np.float32)


def _build_nc():
    import concourse.bass as bass
    import concourse.mybir as mybir
    import concourse.tile as tile
    from concourse import bacc

    f32 = mybir.dt.float32
    f32r = mybir.dt.float32r
    bf = mybir.dt.bfloat16
    Alu = mybir.AluOpType
    Act = mybir.ActivationFunctionType

    nc = bacc.Bacc(None, target_bir_lowering=False, debug=False)

    xTd = nc.dram_tensor("xT", [4, P, KI, 512], bf, kind="ExternalInput")
    xTod = nc.dram_tensor("xTown", [P, KI, NT], bf, kind="ExternalInput")
    xrd = nc.dram_tensor("xrows", [P, NSLOT, D], f32r, kind="ExternalInput")
    Wqd = nc.dram_tensor("Wq", [P, KI, D], bf, kind="ExternalInput")
    Wkd = nc.dram_tensor("Wk", [P, KI, D], bf, kind="ExternalInput")
    Wvd = nc.dram_tensor("Wv", [P, KI, D], bf, kind="ExternalInput")
    Wod = nc.dram_tensor("Wo", [P, KI, D], bf, kind="ExternalInput")
    W1d = nc.dram_tensor("W1", [P, NFT, KI, P], bf, kind="ExternalInput")
    W2d = nc.dram_tensor("W2", [P, NFT, D], bf, kind="ExternalInput")
    cfd = nc.dram_tensor("cf", [P, CF], f32, kind="ExternalInput")
    bcd = nc.dram_tensor("bc", [P, len(BCN) * D], f32, kind="ExternalInput")
    crd = nc.dram_tensor("cr", [P, P], f32r, kind="ExternalInput")
    cbd = nc.dram_tensor("cb", [P, CB], bf, kind="ExternalInput")
    outv = nc.dram_tensor("out", [NT, D], f32, kind="ExternalOutput")

    with tile.TileContext(nc) as tc:
        with (
            tc.tile_pool(name="const", bufs=1) as cst,
            tc.tile_pool(name="wgt", bufs=1) as wgt,
            tc.tile_pool(name="persist", bufs=1) as per,
            tc.tile_pool(name="stream", bufs=2) as stream,
            tc.tile_pool(name="xcs", bufs=1) as xcs,
            tc.tile_pool(name="expbuf", bufs=1) as expbuf,
        ):
            # ---------------- prologue DMAs (need-order) ----------------
            cf = cst.tile([P, CF], f32)
            nc.sync.dma_start(out=cf, in_=cfd[:])
            cr = cst.tile([P, P], f32r)
            nc.sync.dma_start(out=cr, in_=crd[:])
            cb = cst.tile([P, CB], bf)
            nc.sync.dma_start(out=cb, in_=cbd[:])
            xTo = per.tile([P, KI, NT], bf)
            nc.sync.dma_start(out=xTo, in_=xTod[:])
            wq_t = wgt.tile([P, KI, D], bf)
            nc.sync.dma_start(out=wq_t, in_=Wqd[:])
            wk_t = wgt.tile([P, KI, D], bf)
            nc.sync.dma_start(out=wk_t, in_=Wkd[:])
            xc = []
            for ck in range(4):
                xck = xcs.tile([P, KI, 512], bf, tag=f"xc{ck}", name=f"xc{ck}")
                nc.sync.dma_start(out=xck, in_=xTd[ck])
                xc.append(xck)
            wv_t = wgt.tile([P, KI, D], bf)
            nc.sync.dma_start(out=wv_t, in_=Wvd[:])
            bcst = cst.tile([P, len(BCN) * D], f32)
            nc.sync.dma_start(out=bcst, in_=bcd[:])
            xrows = per.tile([P, NSLOT, D], f32r)
            nc.sync.dma_start(out=xrows, in_=xrd[:])
            wo_t = wgt.tile([P, KI, D], bf)
            nc.sync.dma_start(out=wo_t, in_=Wod[:])
            w1_t = wgt.tile([P, NFT, KI, P], bf)
            nc.sync.dma_start(out=w1_t, in_=W1d[:])
            w2_t = wgt.tile([P, NFT, D], bf)
            nc.sync.dma_start(out=w2_t, in_=W2d[:])

            eps_t = cf[:, CF_EPS:CF_EPS + 1]
            bq_t = cf[:, CF_BQ:CF_BQ + DO]
            bk_t = cf[:, CF_BK:CF_BK + DO]
            b1_t = cf[:, CF_B1:CF_B1 + NFT]
            keep_t = cf[:, CF_KEEP:CF_KEEP + NSLOT]
            bc = {n: bcst[:, i * D:(i + 1) * D] for i, n in enumerate(BCN)}
            ident_r = cr[:, 0:P]
            ident_f = ident_r.bitcast(f32)
            identb = cb[:, CB_ID:CB_ID + P]
            maskb = cb[:, CB_MASK:CB_MASK + 512]
            oselb = cb[:, CB_OSEL:CB_OSEL + 32].rearrange(
                "p (o h) -> p o h", o=DO)

            # ---------------- persistent SBUF state ----------------
            qT = per.tile([P, DO, NT], bf)
            kT = per.tile([P, DO, S], bf)
            kTo = per.tile([P, DO, NT], bf)
            v_row = per.tile([P, NSLOT, D], bf)
            qkp = per.tile([P, DO, NT], bf)
            sii_eT = per.tile([H, NT], f32)
            denom = per.tile([P, NSLOT, H], f32)
            rden = per.tile([P, NSLOT, H], f32)
            dp = per.tile([P, NSLOT, H], f32)
            xn1 = per.tile([P, NSLOT, D], bf)
            xnT = per.tile([P, KI, NT], bf)
            xps = per.tile([P, NSLOT, D], f32r)
            out_sb = per.tile([P, NSLOT, D], f32)

            def proj_T(pool, dst, w_t, b_t, src, nm):
                # dst[:, do, :] = (w_t.T @ src) + b  -- [D-part, tok] layout
                for do in range(DO):
                    ps = pool.tile([P, 512], f32, tag="fil", name=nm, bufs=2)
                    for ki in range(KI):
                        nc.tensor.matmul(
                            ps, w_t[:, ki, do * P:(do + 1) * P], src[:, ki, :],
                            start=(ki == 0), stop=(ki == KI - 1))
                    nc.vector.tensor_scalar_add(dst[:, do, :], ps,
                                                b_t[:, do:do + 1])

            def v_block(pool, a):
                ps = pool.tile([P, D], f32, tag="fil", name="fx_v", bufs=2)
                for ki in range(KI):
                    nc.tensor.matmul(
                        ps, xTo[:, ki, a * P:(a + 1) * P], wv_t[:, ki, :],
                        start=(ki == 0), stop=(ki == KI - 1))
                nc.vector.tensor_tensor(v_row[:, a, :], ps, bc["bv"], Alu.add)

            def sii_block(pool, a):
                ps = pool.tile([H, P], f32, tag="psh", name="fx_sii", bufs=1)
                for dt in range(DO):
                    nc.tensor.matmul(ps, oselb[:, dt, :],
                                     qkp[:, dt, a * P:(a + 1) * P],
                                     start=(dt == 0), stop=(dt == DO - 1))
                nc.scalar.activation(sii_eT[:, a * P:(a + 1) * P], ps, Act.Exp)

            def score_head(pool, a, h, nbufs):
                po, pr = (h % 2) * DK, h // 2
                kw = (a + 1) * 512
                ps = pool.tile([P, kw], f32, tag=f"sc{min(a, 2)}",
                               name=f"sc{a}", bufs=nbufs)
                for ck in range(a + 1):
                    nc.tensor.matmul(
                        ps[:, ck * 512:(ck + 1) * 512],
                        qT[po:po + DK, pr, a * P:(a + 1) * P],
                        kT[po:po + DK, pr, ck * 512:(ck + 1) * 512],
                        start=True, stop=(ck < a))
                # staircase causal mask on the last chunk, via PE identity mm
                nc.tensor.matmul(ps[:, a * 512:a * 512 + 512], identb, maskb,
                                 start=False, stop=True)
                esc = expbuf.tile([P, 2048], bf, tag="esc", name="esc", bufs=2)
                nc.scalar.activation(esc[:, :kw], ps, Act.Exp,
                                     accum_out=denom[:, a, h:h + 1])

            def dp_block(pool, a):
                nc.vector.reciprocal(rden[:, a, :], denom[:, a, :])
                ps = pool.tile([P, H], f32, tag="psh", name="sT", bufs=1)
                nc.tensor.matmul(ps, sii_eT[:, a * P:(a + 1) * P],
                                 ident_f[:H, :H],
                                 is_transpose=True, start=True, stop=True)
                nc.vector.tensor_tensor(dp[:, a, :], ps, rden[:, a, :], Alu.mult)
                nc.vector.tensor_scalar_mul(dp[:, a, :], dp[:, a, :],
                                            keep_t[:, a:a + 1])

            def ln_stats(src, rstd, nm):
                st = stream.tile([P, 6], f32, tag="ln_st", name="ln_st")
                nc.vector.bn_stats(out=st, in_=src)
                mv = stream.tile([P, 2], f32, tag="ln_mv", name="ln_mv")
                nc.vector.bn_aggr(out=mv, in_=st)
                # 1/sqrt(var+eps) = exp(-0.5*ln(var+eps)); Ln+Exp live in the
                # same ACT table set as the score exps -> no table reloads.
                lnv = stream.tile([P, 1], f32, tag="lnv", name="lnv")
                nc.scalar.activation(out=lnv, in_=mv[:, 1:2], func=Act.Ln,
                                     bias=eps_t)
                nc.scalar.activation(out=rstd, in_=lnv, func=Act.Exp,
                                     scale=-0.5)
                nc.vector.tensor_scalar(out=nm, in0=mv[:, 0:1], scalar1=rstd,
                                        scalar2=-1.0, op0=Alu.mult,
                                        op1=Alu.mult)

            def attn_ln1(pool, a):
                # wr = dp * v  (bf16), transpose, Wo mm, +residual, LN1
                w = stream.tile([P, D], bf, tag="wr", name="wr")
                nc.vector.tensor_tensor(
                    w.rearrange("p (h d) -> p h d", h=H),
                    v_row[:, a, :].rearrange("p (h d) -> p h d", h=H),
                    dp[:, a, :, None].to_broadcast([P, H, DK]), Alu.mult)
                pw = pool.tile([P, KI, P], bf, tag="work", name="pw", bufs=2)
                for ki in range(KI):
                    nc.tensor.transpose(pw[:, ki, :], w[:, ki * P:(ki + 1) * P],
                                        identb)
                wTs = stream.tile([P, KI, P], bf, tag="wTs", name="wTs")
                nc.vector.tensor_copy(wTs, pw)
                po = pool.tile([P, D], f32, tag="work", name="po", bufs=2)
                for ki in range(KI):
                    nc.tensor.matmul(po, wTs[:, ki, :], wo_t[:, ki, :],
                                     start=(ki == 0), stop=False)
                nc.tensor.matmul(po, ident_r, xrows[:, a, :],
                                 start=False, stop=True)
                rstd = stream.tile([P, 1], f32, tag="r1", name="rstd")
                nm = stream.tile([P, 1], f32, tag="n1", name="nm")
                ln_stats(po, rstd, nm)
                # z (normalized, no gamma -- gamma1 folded into W1 host-side)
                nc.vector.tensor_scalar(out=xn1[:, a, :], in0=po, scalar1=rstd,
                                        scalar2=nm, op0=Alu.mult, op1=Alu.add)
                # xps = gamma1*z + (beta1 + b2)   (residual-2 base, on gpsimd)
                nc.gpsimd.tensor_tensor(xps[:, a, :], xn1[:, a, :], bc["g1"],
                                        Alu.mult)
                nc.gpsimd.tensor_tensor(xps[:, a, :], xps[:, a, :],
                                        bc["b1b2"], Alu.add)
                # transpose xn1 -> xnT for the FFN
                pt = pool.tile([P, KI, P], bf, tag="work", name="pt", bufs=2)
                for ki in range(KI):
                    nc.tensor.transpose(pt[:, ki, :],
                                        xn1[:, a, ki * P:(ki + 1) * P], identb)
                for ki in range(KI):
                    nc.vector.tensor_copy(xnT[:, ki, a * P:(a + 1) * P],
                                          pt[:, ki, :])

            def ffn_ft(pool, half, ft, psy, nb=2):
                # psh = W1[ft-block].T @ xnT(half)   [128 ff x 256 tok]
                psh = pool.tile([P, 256], f32, tag="psh", name="psh", bufs=nb)
                for ki in range(KI):
                    nc.tensor.matmul(psh, w1_t[:, ft, ki, :],
                                     xnT[:, ki, half * 256:(half + 1) * 256],
                                     start=(ki == 0), stop=(ki == KI - 1))
                hr = stream.tile([P, 256], bf, tag="hr", name="hr", bufs=3)
                nc.vector.tensor_scalar(out=hr, in0=psh,
                                        scalar1=b1_t[:, ft:ft + 1],
                                        scalar2=0.0, op0=Alu.add, op1=Alu.max)
                for i in range(2):
                    nc.tensor.matmul(psy[i], hr[:, i * P:(i + 1) * P],
                                     w2_t[:, ft, :],
                                     start=(ft == 0), stop=False)

            def ln2_store(a, psy_a):
                rstd = stream.tile([P, 1], f32, tag="r1", name="rstd2")
                nm = stream.tile([P, 1], f32, tag="n1", name="nm2")
                ln_stats(psy_a, rstd, nm)
                nc.vector.tensor_scalar(out=out_sb[:, a, :], in0=psy_a,
                                        scalar1=rstd, scalar2=nm,
                                        op0=Alu.mult, op1=Alu.add)
                nc.vector.tensor_tensor(out_sb[:, a, :], out_sb[:, a, :],
                                        bc["g2"], Alu.mult)
                nc.gpsimd.tensor_tensor(out_sb[:, a, :], out_sb[:, a, :],
                                        bc["be2"], Alu.add)
                out_re = outv[:].rearrange("(a p) d -> p a d", p=P)
                nc.sync.dma_start(out=out_re[:, a, :], in_=out_sb[:, a, :])

            # ===================== schedule =====================
            # scope A: qT, kT0, slot-0 scores/exps, kTo, kT1, sii0, dp0
            with tc.tile_pool(name="scA", bufs=1, space="PSUM") as sA:
                proj_T(sA, qT, wq_t, bq_t, xTo, "pp_q")
                proj_T(sA, kT[:, :, 0:512], wk_t, bk_t, xc[0], "pp_k0")
                for h in range(4):
                    score_head(sA, 0, h, 2)
                proj_T(sA, kTo, wk_t, bk_t, xTo, "pp_ko")
                for h in range(4, H):
                    score_head(sA, 0, h, 2)
                nc.vector.tensor_tensor(qkp, qT, kTo, Alu.mult)
                sii_block(sA, 0)
                proj_T(sA, kT[:, :, 512:1024], wk_t, bk_t, xc[1], "pp_k1")
                v_block(sA, 0)
                v_block(sA, 1)
                dp_block(sA, 0)

            # scope B: slot-1 scores/exps, kT2, kT3, v23, sii, dp1
            with tc.tile_pool(name="scB", bufs=1, space="PSUM") as sB:
                sii_block(sB, 1)
                for h in range(2):
                    score_head(sB, 1, h, 2)
                proj_T(sB, kT[:, :, 1024:1536], wk_t, bk_t, xc[2], "pp_k2")
                for h in range(2, 6):
                    score_head(sB, 1, h, 2)
                proj_T(sB, kT[:, :, 1536:2048], wk_t, bk_t, xc[3], "pp_k3")
                for h in range(6, H):
                    score_head(sB, 1, h, 2)
                v_block(sB, 2)
                v_block(sB, 3)
                sii_block(sB, 2)
                sii_block(sB, 3)
                dp_block(sB, 1)

            # scope C: slot-2/3 scores/exps interleaved with slot-0/1 tail
            with tc.tile_pool(name="pyC", bufs=1, space="PSUM") as pyC:
                psy01 = [pyC.tile([P, D], f32, tag=f"y{i}", name=f"y{i}")
                         for i in range(2)]
                ft = 0
                with tc.tile_pool(name="scC2", bufs=1, space="PSUM") as sC2:
                    score_head(sC2, 2, 0, 1)
                    attn_ln1(sC2, 0)
                    score_head(sC2, 2, 1, 1)
                    attn_ln1(sC2, 1)
                    for h in range(2, H):
                        score_head(sC2, 2, h, 1)
                        ffn_ft(sC2, 0, ft, psy01, nb=1)
                        ft += 1
                    dp_block(sC2, 2)
                with tc.tile_pool(name="scC3", bufs=1, space="PSUM") as sC3:
                    for h in range(H):
                        score_head(sC3, 3, h, 1)
                        if ft < NFT:
                            ffn_ft(sC3, 0, ft, psy01, nb=1)
                            ft += 1
                    while ft < NFT:
                        ffn_ft(sC3, 0, ft, psy01, nb=1)
                        ft += 1
                    dp_block(sC3, 3)
                    for i in range(2):
                        nc.tensor.matmul(psy01[i], ident_r, xps[:, i, :],
                                         start=False, stop=True)
                        ln2_store(i, psy01[i])

            # scope D: slot-2/3 tail
            with (
                tc.tile_pool(name="scD", bufs=1, space="PSUM") as sD,
                tc.tile_pool(name="pyD", bufs=1, space="PSUM") as pyD,
            ):
                psy23 = [pyD.tile([P, D], f32, tag=f"z{i}", name=f"z{i}")
                         for i in range(2)]
                attn_ln1(sD, 2)
                attn_ln1(sD, 3)
                for ft in range(NFT):
                    ffn_ft(sD, 1, ft, psy23, nb=2)
                for i, a in enumerate((2, 3)):
                    nc.tensor.matmul(psy23[i], ident_r, xps[:, a, :],
                                     start=False, stop=True)
                    ln2_store(a, psy23[i])

    nc.compile()
    return nc


def _get_nc():
    if "nc" not in _CACHE:
        _CACHE["nc"] = _build_nc()
    return _CACHE["nc"]


def _rearr_w(w):
    # [Din, N] -> [P, KI, N] bf16 with [p, o, n] = w[o*128+p, n]
    return np.ascontiguousarray(
        w.astype(bf16).reshape(KI, P, -1).transpose(1, 0, 2))


def kernel(x, lengths, Wq, bq, Wk, bk, Wv, bv, Wo, bo, W1, b1, W2, b2,
           gamma1, beta1, gamma2, beta2):
    global LAST_EXEC_NS
    from concourse.bass_utils import run_bass_kernel_spmd

    x = np.asarray(x, dtype=np.float32)
    lengths = np.asarray(lengths, dtype=np.int32)
    f32a = lambda a: np.asarray(a, dtype=np.float32)

    pad = (np.arange(S)[None, :] < lengths[:, None]).astype(np.float32)
    xm = x * pad[:, :, None]

    g1 = f32a(gamma1)
    W1f = f32a(W1)
    # gamma1/beta1 folded into the FFN: W1' = gamma1*W1, b1' = b1 + beta1@W1
    w1g = g1[:, None] * W1f
    b1p = f32a(b1) + f32a(beta1) @ W1f
    # W1 [D, FF] -> [P, NFT, KI, P]: [p, ft, ki, m] = w1g[ki*128+p, ft*128+m]
    w1p = np.ascontiguousarray(
        w1g.astype(bf16).reshape(KI, P, NFT, P).transpose(1, 2, 0, 3))
    # W2 [FF, D] -> [P, NFT, D]: [p, ft, n] = W2[ft*128+p, n]
    w2p = np.ascontiguousarray(
        f32a(W2).astype(bf16).reshape(NFT, P, D).transpose(1, 0, 2))

    cfv = np.zeros((P, CF), dtype=np.float32)
    cfv[:, CF_EPS] = EPS
    cfv[:, CF_BQ:CF_BQ + DO] = f32a(bq).reshape(DO, P).T
    cfv[:, CF_BK:CF_BK + DO] = f32a(bk).reshape(DO, P).T
    cfv[:, CF_B1:CF_B1 + NFT] = b1p.reshape(NFT, P).T
    b1b2 = f32a(beta1) + f32a(b2)
    bcv = np.zeros((P, len(BCN) * D), dtype=np.float32)
    for i, v in enumerate([f32a(bv), g1, b1b2, f32a(gamma2), f32a(beta2)]):
        bcv[:, i * D:(i + 1) * D] = v[None, :]

    osel = np.zeros((P, DO, H), dtype=np.float32)
    for dt in range(DO):
        osel[:DK, dt, 2 * dt] = 1.0
        osel[DK:, dt, 2 * dt + 1] = 1.0

    crv = to_f32r(np.eye(P, dtype=np.float32))

    cols = np.arange(512)[None, :]
    rows = np.arange(P)[:, None]

    common = dict(Wq=_rearr_w(f32a(Wq)), Wk=_rearr_w(f32a(Wk)),
                  Wv=_rearr_w(f32a(Wv)), Wo=_rearr_w(f32a(Wo)),
                  W1=w1p, W2=w2p, cr=crv, bc=bcv)

    bo_f = f32a(bo)
    in_maps = []
    for c in range(8):
        b, p = c // 4, c % 4
        xTb = np.ascontiguousarray(xm[b].T).astype(bf16)      # [D, S]
        xt4 = np.ascontiguousarray(
            xTb.reshape(KI, P, 4, 512).transpose(2, 1, 0, 3))
        xto = np.ascontiguousarray(
            xTb[:, p::4].reshape(KI, P, NT).transpose(1, 0, 2))
        # xrows: own tokens row-layout + bo, f32r: [r, a, d]
        xr = np.ascontiguousarray(
            to_f32r(xm[b, p::4, :] + bo_f[None, :]).reshape(NSLOT, P, D)
            .transpose(1, 0, 2))
        m = np.where(cols <= 4 * rows + p, 0.0, NEG).astype(bf16)
        tloc = p + 4 * (np.arange(NSLOT)[None, :] * P + rows)
        keepm = (tloc < lengths[b]).astype(np.float32)
        cfc = cfv.copy()
        cfc[:, CF_KEEP:CF_KEEP + NSLOT] = keepm
        cbv = np.zeros((P, CB), dtype=bf16)
        cbv[:, CB_ID:CB_ID + P] = np.eye(P, dtype=bf16)
        cbv[:, CB_MASK:CB_MASK + 512] = m
        cbv[:, CB_OSEL:CB_OSEL + 32] = osel.reshape(P, 32).astype(bf16)
        in_maps.append(dict(xT=xt4, xTown=xto, xrows=xr, cf=cfc, cb=cbv,
                            **common))

    nc = _get_nc()
    res = run_bass_kernel_spmd(nc, in_maps, list(range(8)), trace=TRACE)
    LAST_EXEC_NS = res.exec_time_ns

    out = np.empty((B, S, D), dtype=np.float32)
    for c in range(8):
        b, p = c // 4, c % 4
        out[b, p::4, :] = res.results[c]["out"]
    return out


# revision 7
# speedup vs baseline: 1.2727x; 1.2727x over previous
"""Trainium2 Bass kernel for nn_DecoderBlock_85761906966851 (v2, pipelined).

The reference decoder block's attention einsum ('bhss,bshd->bshd') takes the
DIAGONAL of the attention matrix, so token i only needs
    diag_prob_i[h] = exp(s_ii) / sum_{j<=i} exp(s_ij)
per head.  The kernel computes causal row-sums of exp(QK^T) (fused
exp+row-accumulate on the scalar engine), diagonal scores via an elementwise
q*k reduction, then a dense per-token tail (Wo projection, LayerNorm, FFN,
LayerNorm).

v2 restructure vs baseline:
  * Software-pipelined: the per-token tail for token-slots 0-1 (Wo, LN1,
    FFN, LN2, store) is interleaved into the exp-bound score phases of
    slots 2-3, so the PE works while ACT grinds exps.
  * bf16 matmul operands everywhere except the residual path (f32r),
    halving DMA bytes and enabling FWL weight loads.
  * Staircase causal mask added in PSUM by an identity matmul on the PE
    (frees the DVE); single exp per (slot, head) spanning all key chunks.
  * LayerNorm rsqrt via exp(-0.5*ln(var+eps)) -- Ln/Exp/Identity live in
    one ACT table set, so zero table reloads even with LNs interleaved
    between score exps.
  * gamma1/beta1/b1 folded into W1/b1 host-side; x+bo residual pre-added
    host-side and DMA'd in row layout (no PE transposes for it).

Sharding: 8 cores = 2 batches x 4 stride offsets; core (b, p) owns tokens
p::4 of batch b (the interleave equalizes causal work).  No collectives;
k is recomputed per core.
"""

import numpy as np
import ml_dtypes

B, S, D, H, FF = 2, 2048, 512, 8, 2048
DK = D // H          # 64
P = 128
NT = 512             # tokens per core
NSLOT = 4
DO = D // P          # 4
KI = D // P          # 4
NFT = FF // P        # 16
EPS = 1e-3
NEG = -1.0e30

bf16 = ml_dtypes.bfloat16

# packed f32 consts: eps | bq(4) | bk(4) | b1(16) | keep(4)
CF_EPS, CF_BQ, CF_BK, CF_B1, CF_KEEP = 0, 1, 5, 9, 25
CF = 29
# broadcast f32 consts [P, 5*D]
BCN = ["bv", "g1", "b1b2", "g2", "be2"]
# bf16 consts: identb(128) | mask(512) | osel(32)
CB_ID, CB_MASK, CB_OSEL = 0, 128, 640
CB = 672

TRACE = False
LAST_EXEC_NS = None
_CACHE = {}


def to_f32r(a):
    """Round fp32 to fp32r (11-bit mantissa, round half up at bit 12)."""
    b = np.ascontiguousarray(a, dtype=np.float32).view(np.uint32)
    r = ((b.astype(np.uint64) + 0x800) & 0xFFFFF000).astype(np.uint32)
    return r.view(np.float32)


def _build_nc():
    import concourse.bass as bass
    import concourse.mybir as mybir
    import concourse.tile as tile
    from concourse import bacc

    f32 = mybir.dt.float32
    f32r = mybir.dt.float32r
    bf = mybir.dt.bfloat16
    Alu = mybir.AluOpType
    Act = mybir.ActivationFunctionType

    nc = bacc.Bacc(None, target_bir_lowering=False, debug=False)

    xTd = nc.dram_tensor("xT", [4, P, KI, 512], bf, kind="ExternalInput")
    xTod = nc.dram_tensor("xTown", [P, KI, NT], bf, kind="ExternalInput")
    xrd = nc.dram_tensor("xrows", [P, NSLOT, D], f32r, kind="ExternalInput")
    Wqd = nc.dram_tensor("Wq", [P, KI, D], bf, kind="ExternalInput")
    Wkd = nc.dram_tensor("Wk", [P, KI, D], bf, kind="ExternalInput")
    Wvd = nc.dram_tensor("Wv", [P, KI, D], bf, kind="ExternalInput")
    Wod = nc.dram_tensor("Wo", [P, KI, D], bf, kind="ExternalInput")
    W1d = nc.dram_tensor("W1", [P, NFT, KI, P], bf, kind="ExternalInput")
    W2d = nc.dram_tensor("W2", [P, NFT, D], bf, kind="ExternalInput")
    cfd = nc.dram_tensor("cf", [P, CF], f32, kind="ExternalInput")
    bcd = nc.dram_tensor("bc", [P, len(BCN) * D], f32, kind="ExternalInput")
    crd = nc.dram_tensor("cr", [P, P], f32r, kind="ExternalInput")
    cbd = nc.dram_tensor("cb", [P, CB], bf, kind="ExternalInput")
    outv = nc.dram_tensor("out", [NT, D], f32, kind="ExternalOutput")

    with tile.TileContext(nc) as tc:
        with (
            tc.tile_pool(name="const", bufs=1) as cst,
            tc.tile_pool(name="wgt", bufs=1) as wgt,
            tc.tile_pool(name="persist", bufs=1) as per,
            tc.tile_pool(name="stream", bufs=2) as stream,
            tc.tile_pool(name="xcs", bufs=1) as xcs,
            tc.tile_pool(name="expbuf", bufs=1) as expbuf,
        ):
            # ---------------- prologue DMAs (need-order) ----------------
            cf = cst.tile([P, CF], f32)
            nc.sync.dma_start(out=cf, in_=cfd[:])
            cr = cst.tile([P, P], f32r)
            nc.sync.dma_start(out=cr, in_=crd[:])
            cb = cst.tile([P, CB], bf)
            nc.sync.dma_start(out=cb, in_=cbd[:])
            xTo = per.tile([P, KI, NT], bf)
            nc.sync.dma_start(out=xTo, in_=xTod[:])
            wq_t = wgt.tile([P, KI, D], bf)
            nc.sync.dma_start(out=wq_t, in_=Wqd[:])
            wk_t = wgt.tile([P, KI, D], bf)
            nc.sync.dma_start(out=wk_t, in_=Wkd[:])
            xc = []
            for ck in range(4):
                xck = xcs.tile([P, KI, 512], bf, tag=f"xc{ck}", name=f"xc{ck}")
                nc.sync.dma_start(out=xck, in_=xTd[ck])
                xc.append(xck)
            wv_t = wgt.tile([P, KI, D], bf)
            nc.sync.dma_start(out=wv_t, in_=Wvd[:])
            bcst = cst.tile([P, len(BCN) * D], f32)
            nc.sync.dma_start(out=bcst, in_=bcd[:])
            xrows = per.tile([P, NSLOT, D], f32r)
            nc.sync.dma_start(out=xrows, in_=xrd[:])
            wo_t = wgt.tile([P, KI, D], bf)
            nc.sync.dma_start(out=wo_t, in_=Wod[:])
            w1_t = wgt.tile([P, NFT, KI, P], bf)
            nc.sync.dma_start(out=w1_t, in_=W1d[:])
            w2_t = wgt.tile([P, NFT, D], bf)
            nc.sync.dma_start(out=w2_t, in_=W2d[:])

            eps_t = cf[:, CF_EPS:CF_EPS + 1]
            bq_t = cf[:, CF_BQ:CF_BQ + DO]
            bk_t = cf[:, CF_BK:CF_BK + DO]
            b1_t = cf[:, CF_B1:CF_B1 + NFT]
            keep_t = cf[:, CF_KEEP:CF_KEEP + NSLOT]
            bc = {n: bcst[:, i * D:(i + 1) * D] for i, n in enumerate(BCN)}
            ident_r = cr[:, 0:P]
            ident_f = ident_r.bitcast(f32)
            identb = cb[:, CB_ID:CB_ID + P]
            maskb = cb[:, CB_MASK:CB_MASK + 512]
            oselb = cb[:, CB_OSEL:CB_OSEL + 32].rearrange(
                "p (o h) -> p o h", o=DO)

            # ---------------- persistent SBUF state ----------------
            qT = per.tile([P, DO, NT], bf)
            kT = per.tile([P, DO, S], bf)
            kTo = per.tile([P, DO, NT], bf)
            v_row = per.tile([P, NSLOT, D], bf)
            qkp = per.tile([P, DO, NT], bf)
            sii_eT = per.tile([H, NT], f32)
            denom = per.tile([P, NSLOT, H], f32)
            rden = per.tile([P, NSLOT, H], f32)
            dp = per.tile([P, NSLOT, H], f32)
            xn1 = per.tile([P, NSLOT, D], bf)
            xnT = per.tile([P, KI, NT], bf)
            xps = per.tile([P, NSLOT, D], f32r)
            out_sb = per.tile([P, NSLOT, D], f32)

            def proj_T(pool, dst, w_t, b_t, src, nm):
                # dst[:, do, :] = (w_t.T @ src) + b  -- [D-part, tok] layout
                for do in range(DO):
                    ps = pool.tile([P, 512], f32, tag="fil", name=nm, bufs=2)
                    for ki in range(KI):
                        nc.tensor.matmul(
                            ps, w_t[:, ki, do * P:(do + 1) * P], src[:, ki, :],
                            start=(ki == 0), stop=(ki == KI - 1))
                    nc.vector.tensor_scalar_add(dst[:, do, :], ps,
                                                b_t[:, do:do + 1])

            def v_block(pool, a):
                ps = pool.tile([P, D], f32, tag="fil", name="fx_v", bufs=2)
                for ki in range(KI):
                    nc.tensor.matmul(
                        ps, xTo[:, ki, a * P:(a + 1) * P], wv_t[:, ki, :],
                        start=(ki == 0), stop=(ki == KI - 1))
                nc.vector.tensor_tensor(v_row[:, a, :], ps, bc["bv"], Alu.add)

            def sii_block(pool, a):
                ps = pool.tile([H, P], f32, tag="psh", name="fx_sii", bufs=1)
                for dt in range(DO):
                    nc.tensor.matmul(ps, oselb[:, dt, :],
                                     qkp[:, dt, a * P:(a + 1) * P],
                                     start=(dt == 0), stop=(dt == DO - 1))
                nc.scalar.activation(sii_eT[:, a * P:(a + 1) * P], ps, Act.Exp)

            def score_head(pool, a, h, nbufs):
                po, pr = (h % 2) * DK, h // 2
                kw = (a + 1) * 512
                ps = pool.tile([P, kw], f32, tag=f"sc{min(a, 2)}",
                               name=f"sc{a}", bufs=nbufs)
                for ck in range(a + 1):
                    nc.tensor.matmul(
                        ps[:, ck * 512:(ck + 1) * 512],
                        qT[po:po + DK, pr, a * P:(a + 1) * P],
                        kT[po:po + DK, pr, ck * 512:(ck + 1) * 512],
                        start=True, stop=(ck < a), tile_position=(po, 0))
                # staircase causal mask on the last chunk, via PE identity mm
                nc.tensor.matmul(ps[:, a * 512:a * 512 + 512], identb, maskb,
                                 start=False, stop=True)
                esc = expbuf.tile([P, 2048], bf, tag="esc", name="esc", bufs=2)
                nc.scalar.activation(esc[:, :kw], ps, Act.Exp,
                                     accum_out=denom[:, a, h:h + 1])

            def dp_block(pool, a):
                nc.vector.reciprocal(rden[:, a, :], denom[:, a, :])
                ps = pool.tile([P, H], f32, tag="psh", name="sT", bufs=1)
                nc.tensor.matmul(ps, sii_eT[:, a * P:(a + 1) * P],
                                 ident_f[:H, :H],
                                 is_transpose=True, start=True, stop=True)
                nc.vector.tensor_tensor(dp[:, a, :], ps, rden[:, a, :], Alu.mult)
                nc.vector.tensor_scalar_mul(dp[:, a, :], dp[:, a, :],
                                            keep_t[:, a:a + 1])

            i32 = mybir.dt.int32

            def ln_stats(src, rstd, nm):
                # mean/var via bn_stats, then rsqrt(var+eps) fully on the DVE
                # (quake-III seed + 2 Newton steps) -- keeps ACT exp-only, so
                # the exp table set is loaded exactly once for the kernel.
                st = stream.tile([P, 6], f32, tag="ln_st", name="ln_st")
                nc.vector.bn_stats(out=st, in_=src)
                mv = stream.tile([P, 2], f32, tag="ln_mv", name="ln_mv")
                nc.vector.bn_aggr(out=mv, in_=st)
                w = stream.tile([P, 4], f32, tag="ln_w", name="ln_w")
                v_ = w[:, 0:1]
                y = w[:, 1:2]
                t = w[:, 2:3]
                nc.vector.tensor_scalar(out=v_, in0=mv[:, 1:2], scalar1=EPS,
                                        scalar2=None, op0=Alu.add)
                nc.vector.tensor_scalar(
                    out=y.bitcast(i32), in0=v_.bitcast(i32), scalar1=1,
                    scalar2=None, op0=Alu.logical_shift_right)
                nc.vector.tensor_scalar(
                    out=y.bitcast(i32), in0=y.bitcast(i32), scalar1=-1,
                    scalar2=0x5F3759DF, op0=Alu.mult, op1=Alu.add)
                for _ in range(2):
                    nc.vector.tensor_tensor(t, y, y, Alu.mult)
                    nc.vector.tensor_tensor(t, t, v_, Alu.mult)
                    nc.vector.tensor_scalar(out=t, in0=t, scalar1=-0.5,
                                            scalar2=1.5, op0=Alu.mult,
                                            op1=Alu.add)
                    nc.vector.tensor_tensor(y, y, t, Alu.mult)
                nc.vector.tensor_copy(rstd, y)
                nc.vector.tensor_scalar(out=nm, in0=mv[:, 0:1], scalar1=rstd,
                                        scalar2=-1.0, op0=Alu.mult,
                                        op1=Alu.mult)

            def attn_ln1(pool, a):
                # wr = dp * v  (bf16), transpose, Wo mm, +residual, LN1
                w = stream.tile([P, D], bf, tag="wr", name="wr")
                nc.vector.tensor_tensor(
                    w.rearrange("p (h d) -> p h d", h=H),
                    v_row[:, a, :].rearrange("p (h d) -> p h d", h=H),
                    dp[:, a, :, None].to_broadcast([P, H, DK]), Alu.mult)
                pw = pool.tile([P, KI, P], bf, tag="work", name="pw", bufs=2)
                for ki in range(KI):
                    nc.tensor.transpose(pw[:, ki, :], w[:, ki * P:(ki + 1) * P],
                                        identb)
                wTs = stream.tile([P, KI, P], bf, tag="wTs", name="wTs")
                nc.vector.tensor_copy(wTs, pw)
                po = pool.tile([P, D], f32, tag="work", name="po", bufs=2)
                for ki in range(KI):
                    nc.tensor.matmul(po, wTs[:, ki, :], wo_t[:, ki, :],
                                     start=(ki == 0), stop=False)
                nc.tensor.matmul(po, ident_r, xrows[:, a, :],
                                 start=False, stop=True)
                rstd = stream.tile([P, 1], f32, tag="r1", name="rstd")
                nm = stream.tile([P, 1], f32, tag="n1", name="nm")
                ln_stats(po, rstd, nm)
                # z (normalized, no gamma -- gamma1 folded into W1 host-side)
                nc.vector.tensor_scalar(out=xn1[:, a, :], in0=po, scalar1=rstd,
                                        scalar2=nm, op0=Alu.mult, op1=Alu.add)
                # xps = gamma1*z + (beta1 + b2)   (residual-2 base, on gpsimd)
                nc.gpsimd.tensor_tensor(xps[:, a, :], xn1[:, a, :], bc["g1"],
                                        Alu.mult)
                nc.gpsimd.tensor_tensor(xps[:, a, :], xps[:, a, :],
                                        bc["b1b2"], Alu.add)
                # transpose xn1 -> xnT for the FFN
                pt = pool.tile([P, KI, P], bf, tag="work", name="pt", bufs=2)
                for ki in range(KI):
                    nc.tensor.transpose(pt[:, ki, :],
                                        xn1[:, a, ki * P:(ki + 1) * P], identb)
                for ki in range(KI):
                    nc.vector.tensor_copy(xnT[:, ki, a * P:(a + 1) * P],
                                          pt[:, ki, :])

            def ffn_ft(pool, half, ft, psy, nb=2):
                # psh = W1[ft-block].T @ xnT(half)   [128 ff x 256 tok]
                psh = pool.tile([P, 256], f32, tag="psh", name="psh", bufs=nb)
                for ki in range(KI):
                    nc.tensor.matmul(psh, w1_t[:, ft, ki, :],
                                     xnT[:, ki, half * 256:(half + 1) * 256],
                                     start=(ki == 0), stop=(ki == KI - 1))
                hr = stream.tile([P, 256], bf, tag="hr", name="hr", bufs=3)
                nc.vector.tensor_scalar(out=hr, in0=psh,
                                        scalar1=b1_t[:, ft:ft + 1],
                                        scalar2=0.0, op0=Alu.add, op1=Alu.max)
                for i in range(2):
                    nc.tensor.matmul(psy[i], hr[:, i * P:(i + 1) * P],
                                     w2_t[:, ft, :],
                                     start=(ft == 0), stop=False)

            def ln2_store(a, psy_a):
                rstd = stream.tile([P, 1], f32, tag="r1", name="rstd2")
                nm = stream.tile([P, 1], f32, tag="n1", name="nm2")
                ln_stats(psy_a, rstd, nm)
                nc.vector.tensor_scalar(out=out_sb[:, a, :], in0=psy_a,
                                        scalar1=rstd, scalar2=nm,
                                        op0=Alu.mult, op1=Alu.add)
                nc.vector.tensor_tensor(out_sb[:, a, :], out_sb[:, a, :],
                                        bc["g2"], Alu.mult)
                nc.gpsimd.tensor_tensor(out_sb[:, a, :], out_sb[:, a, :],
                                        bc["be2"], Alu.add)
                out_re = outv[:].rearrange("(a p) d -> p a d", p=P)
                nc.sync.dma_start(out=out_re[:, a, :], in_=out_sb[:, a, :])

            # ===================== schedule =====================
            # scope A: qT, kT0, slot-0 scores/exps, kTo, kT1, sii0, dp0
            with tc.tile_pool(name="scA", bufs=1, space="PSUM") as sA:
                proj_T(sA, qT, wq_t, bq_t, xTo, "pp_q")
                proj_T(sA, kT[:, :, 0:512], wk_t, bk_t, xc[0], "pp_k0")
                for h in range(4):
                    score_head(sA, 0, h, 2)
                proj_T(sA, kTo, wk_t, bk_t, xTo, "pp_ko")
                for h in range(4, H):
                    score_head(sA, 0, h, 2)
                nc.vector.tensor_tensor(qkp, qT, kTo, Alu.mult)
                sii_block(sA, 0)
                proj_T(sA, kT[:, :, 512:1024], wk_t, bk_t, xc[1], "pp_k1")
                v_block(sA, 0)
                v_block(sA, 1)
                dp_block(sA, 0)

            # scope B: slot-1 scores/exps, kT2, kT3, v23, sii, dp1
            with tc.tile_pool(name="scB", bufs=1, space="PSUM") as sB:
                sii_block(sB, 1)
                for h in range(2):
                    score_head(sB, 1, h, 2)
                proj_T(sB, kT[:, :, 1024:1536], wk_t, bk_t, xc[2], "pp_k2")
                for h in range(2, 6):
                    score_head(sB, 1, h, 2)
                proj_T(sB, kT[:, :, 1536:2048], wk_t, bk_t, xc[3], "pp_k3")
                for h in range(6, H):
                    score_head(sB, 1, h, 2)
                v_block(sB, 2)
                v_block(sB, 3)
                sii_block(sB, 2)
                sii_block(sB, 3)
                dp_block(sB, 1)

            # scope C: slot-2/3 scores/exps interleaved with slot-0/1 tail
            with tc.tile_pool(name="pyC", bufs=1, space="PSUM") as pyC:
                psy01 = [pyC.tile([P, D], f32, tag=f"y{i}", name=f"y{i}")
                         for i in range(2)]
                ft = 0
                with tc.tile_pool(name="scC2", bufs=1, space="PSUM") as sC2:
                    score_head(sC2, 2, 0, 1)
                    attn_ln1(sC2, 0)
                    score_head(sC2, 2, 1, 1)
                    attn_ln1(sC2, 1)
                    for h in range(2, H):
                        score_head(sC2, 2, h, 1)
                        ffn_ft(sC2, 0, ft, psy01, nb=1)
                        ft += 1
                    dp_block(sC2, 2)
                with tc.tile_pool(name="scC3", bufs=1, space="PSUM") as sC3:
                    for h in range(H):
                        score_head(sC3, 3, h, 1)
                        if ft < NFT:
                            ffn_ft(sC3, 0, ft, psy01, nb=1)
                            ft += 1
                    while ft < NFT:
                        ffn_ft(sC3, 0, ft, psy01, nb=1)
                        ft += 1
                    dp_block(sC3, 3)
                    for i in range(2):
                        nc.tensor.matmul(psy01[i], ident_r, xps[:, i, :],
                                         start=False, stop=True)
                        ln2_store(i, psy01[i])

            # scope D: slot-2/3 tail
            with (
                tc.tile_pool(name="scD", bufs=1, space="PSUM") as sD,
                tc.tile_pool(name="pyD", bufs=1, space="PSUM") as pyD,
            ):
                psy23 = [pyD.tile([P, D], f32, tag=f"z{i}", name=f"z{i}")
                         for i in range(2)]
                attn_ln1(sD, 2)
                attn_ln1(sD, 3)
                for ft in range(NFT):
                    ffn_ft(sD, 1, ft, psy23, nb=2)
                for i, a in enumerate((2, 3)):
                    nc.tensor.matmul(psy23[i], ident_r, xps[:, a, :],
                                     start=False, stop=True)
                    ln2_store(a, psy23[i])

    nc.compile()
    return nc


def _get_nc():
    if "nc" not in _CACHE:
        _CACHE["nc"] = _build_nc()
    return _CACHE["nc"]


def _rearr_w(w):
    # [Din, N] -> [P, KI, N] bf16 with [p, o, n] = w[o*128+p, n]
    return np.ascontiguousarray(
        w.astype(bf16).reshape(KI, P, -1).transpose(1, 0, 2))


def kernel(x, lengths, Wq, bq, Wk, bk, Wv, bv, Wo, bo, W1, b1, W2, b2,
           gamma1, beta1, gamma2, beta2):
    global LAST_EXEC_NS
    from concourse.bass_utils import run_bass_kernel_spmd

    x = np.asarray(x, dtype=np.float32)
    lengths = np.asarray(lengths, dtype=np.int32)
    f32a = lambda a: np.asarray(a, dtype=np.float32)

    pad = (np.arange(S)[None, :] < lengths[:, None]).astype(np.float32)
    xm = x * pad[:, :, None]

    g1 = f32a(gamma1)
    W1f = f32a(W1)
    # gamma1/beta1 folded into the FFN: W1' = gamma1*W1, b1' = b1 + beta1@W1
    w1g = g1[:, None] * W1f
    b1p = f32a(b1) + f32a(beta1) @ W1f
    # W1 [D, FF] -> [P, NFT, KI, P]: [p, ft, ki, m] = w1g[ki*128+p, ft*128+m]
    w1p = np.ascontiguousarray(
        w1g.astype(bf16).reshape(KI, P, NFT, P).transpose(1, 2, 0, 3))
    # W2 [FF, D] -> [P, NFT, D]: [p, ft, n] = W2[ft*128+p, n]
    w2p = np.ascontiguousarray(
        f32a(W2).astype(bf16).reshape(NFT, P, D).transpose(1, 0, 2))

    cfv = np.zeros((P, CF), dtype=np.float32)
    cfv[:, CF_EPS] = EPS
    cfv[:, CF_BQ:CF_BQ + DO] = f32a(bq).reshape(DO, P).T
    cfv[:, CF_BK:CF_BK + DO] = f32a(bk).reshape(DO, P).T
    cfv[:, CF_B1:CF_B1 + NFT] = b1p.reshape(NFT, P).T
    b1b2 = f32a(beta1) + f32a(b2)
    bcv = np.zeros((P, len(BCN) * D), dtype=np.float32)
    for i, v in enumerate([f32a(bv), g1, b1b2, f32a(gamma2), f32a(beta2)]):
        bcv[:, i * D:(i + 1) * D] = v[None, :]

    osel = np.zeros((P, DO, H), dtype=np.float32)
    for dt in range(DO):
        osel[:DK, dt, 2 * dt] = 1.0
        osel[DK:, dt, 2 * dt + 1] = 1.0

    crv = to_f32r(np.eye(P, dtype=np.float32))

    cols = np.arange(512)[None, :]
    rows = np.arange(P)[:, None]

    common = dict(Wq=_rearr_w(f32a(Wq)), Wk=_rearr_w(f32a(Wk)),
                  Wv=_rearr_w(f32a(Wv)), Wo=_rearr_w(f32a(Wo)),
                  W1=w1p, W2=w2p, cr=crv, bc=bcv)

    bo_f = f32a(bo)
    in_maps = []
    for c in range(8):
        b, p = c // 4, c % 4
        xTb = np.ascontiguousarray(xm[b].T).astype(bf16)      # [D, S]
        xt4 = np.ascontiguousarray(
            xTb.reshape(KI, P, 4, 512).transpose(2, 1, 0, 3))
        xto = np.ascontiguousarray(
            xTb[:, p::4].reshape(KI, P, NT).transpose(1, 0, 2))
        # xrows: own tokens row-layout + bo, f32r: [r, a, d]
        xr = np.ascontiguousarray(
            to_f32r(xm[b, p::4, :] + bo_f[None, :]).reshape(NSLOT, P, D)
            .transpose(1, 0, 2))
        m = np.where(cols <= 4 * rows + p, 0.0, NEG).astype(bf16)
        tloc = p + 4 * (np.arange(NSLOT)[None, :] * P + rows)
        keepm = (tloc < lengths[b]).astype(np.float32)
        cfc = cfv.copy()
        cfc[:, CF_KEEP:CF_KEEP + NSLOT] = keepm
        cbv = np.zeros((P, CB), dtype=bf16)
        cbv[:, CB_ID:CB_ID + P] = np.eye(P, dtype=bf16)
        cbv[:, CB_MASK:CB_MASK + 512] = m
        cbv[:, CB_OSEL:CB_OSEL + 32] = osel.reshape(P, 32).astype(bf16)
        in_maps.append(dict(xT=xt4, xTown=xto, xrows=xr, cf=cfc, cb=cbv,
                            **common))

    nc = _get_nc()
    res = run_bass_kernel_spmd(nc, in_maps, list(range(8)), trace=TRACE)
    LAST_EXEC_NS = res.exec_time_ns

    out = np.empty((B, S, D), dtype=np.float32)
    for c in range(8):
        b, p = c // 4, c % 4
        out[b, p::4, :] = res.results[c]["out"]
    return out
